# revision 3
# baseline (speedup 1.0000x reference)
"""GCN 3-layer message-passing kernel for Trainium2 (8 NeuronCores).

Strategy
--------
Nodes are sharded across the 8 cores by destination (graph parallel), with a
degree-balanced permutation so every 64-node "group" has a near-equal edge
count. Per layer, each core:
  1. gathers source-node feature rows (f32, 512B) from its local full copy of
     G = H @ W with `dma_gather` (single-packet, 1024 idx/call, 4 SWDGE
     queues — descriptor generation on the Q7 cores is the bottleneck),
  2. aggregates 128-edge chunks into PSUM via matmul against a host-built
     one-hot matrix carrying the symmetric norm (out = Msg.T @ OneHot,
     feature-major [128f x 64dst] accumulators),
  3. applies bias + LeakyReLU on the scalar engine (Lrelu activation),
  4. computes the next layer's G tiles node-major with a stationary trick
     (lhsT = H^T tile, rhs = W) and AllGathers the slice to every core.
The final layer multiplies by identity instead of W to emit node-major f32
output rows.
"""

import sys

for _p in ("/opt/trn_rl_repo", "/root/.axon_site"):
    if _p not in sys.path:
        sys.path.insert(0, _p)

import numpy as np

P = 128
DW = 64            # dst nodes per PSUM group
NCORES = 8
NBLK = 6400        # dst slots per core (100 groups of 64)
NG = NBLK // DW    # 100 groups per core
SLAB_G = 4         # groups per slab (gather/onehot batch)
NSLAB = NG // SLAB_G
NTOT = NBLK * NCORES  # 51200 slots
SPLIT = 32768      # int16 gather index split
GSZ = 1024         # idxs per dma_gather (single-packet limit)
ALPHA = 0.01
NLAYER = 3


def _even_ceil(n, m):
    c = -(-n // m)
    return c + (c % 2)


def _preprocess(x, edge_index):
    """Host-side graph preprocessing -> per-core idx tables, onehot, layout."""
    n_nodes = x.shape[0]
    src = np.concatenate([edge_index[0], np.arange(n_nodes, dtype=np.int64)])
    dst = np.concatenate([edge_index[1], np.arange(n_nodes, dtype=np.int64)])
    deg = np.bincount(dst, minlength=n_nodes).astype(np.float64)
    dinv = np.where(deg > 0, deg**-0.5, 0.0)
    norm = (dinv[src] * dinv[dst]).astype(np.float32)

    # --- degree-balanced node -> slot assignment (snake deal into groups) ---
    n_groups_tot = NTOT // DW  # 800
    order = np.argsort(-deg, kind="stable")  # high degree first
    group_of = np.empty(NTOT, dtype=np.int64)  # by deal position
    pos = np.arange(NTOT)
    row = pos // n_groups_tot
    col = pos % n_groups_tot
    group_of = np.where(row % 2 == 0, col, n_groups_tot - 1 - col)
    # node order[i] -> group group_of[i]; remaining slots go to pad nodes
    slot = np.full(NTOT, -1, dtype=np.int64)  # slot -> node (-1 pad)
    node_slot = np.empty(n_nodes, dtype=np.int64)
    fill = np.zeros(n_groups_tot, dtype=np.int64)
    g_arr = group_of[:n_nodes]
    # sequential fill within groups (vector friendly: argsort by group, stable)
    order_by_group = np.argsort(g_arr, kind="stable")
    gs = g_arr[order_by_group]
    within = np.arange(n_nodes) - np.searchsorted(gs, gs)
    slots_for = gs * DW + within
    node_slot[order[order_by_group]] = slots_for
    slot[slots_for] = order[order_by_group]

    s_src = node_slot[src]
    s_dst = node_slot[dst]
    is_high = s_src >= SPLIT
    core = s_dst // NBLK
    group_g = s_dst // DW  # global group id
    dst_local = s_dst % DW

    # per (group, range) edge counts -> uniform CL/CH chunk counts
    key = group_g * 2 + is_high
    counts = np.bincount(key, minlength=n_groups_tot * 2)
    low_max = counts[0::2].max()
    high_max = counts[1::2].max()
    CL = _even_ceil(int(low_max), P)
    CH = _even_ceil(int(high_max), P)
    assert (SLAB_G * CL * P) % GSZ == 0 and (SLAB_G * CH * P) % GSZ == 0
    CPG = CL + CH
    nchunk_core = NG * CPG

    # order edges by (group, range, arbitrary) and compute padded positions
    sort_key = key
    eorder = np.argsort(sort_key, kind="stable")
    s_src_s = s_src[eorder]
    is_high_s = is_high[eorder]
    group_s = group_g[eorder]
    dstl_s = dst_local[eorder]
    norm_s = norm[eorder]
    # position within its (group, range) bucket
    k_s = sort_key[eorder]
    bucket_start = np.searchsorted(k_s, k_s)
    within_b = np.arange(len(k_s)) - bucket_start

    # padded slot id of each edge inside its core's layer-stream
    # stream layout per core: slab-major; per slab: 4 groups' low chunks
    # (each CL*128), then 4 groups' high chunks (each CH*128)
    g_in_core = group_s % (NG)
    slab = g_in_core // SLAB_G
    g_in_slab = g_in_core % SLAB_G
    slab_base = slab * SLAB_G * CPG * P
    off_range = np.where(is_high_s, SLAB_G * CL * P + g_in_slab * CH * P,
                         g_in_slab * CL * P)
    pos_in_stream = slab_base + off_range + within_b
    core_s = group_s // NG

    n_stream = nchunk_core * P
    idx_all = np.zeros((NCORES, n_stream), dtype=np.int16)
    oh_all = np.zeros((NCORES, n_stream), dtype=np.float32)  # norm per slot
    dstl_all = np.zeros((NCORES, n_stream), dtype=np.int64)
    gval = np.where(is_high_s, s_src_s - SPLIT, s_src_s).astype(np.int16)
    idx_all[core_s, pos_in_stream] = gval
    oh_all[core_s, pos_in_stream] = norm_s
    dstl_all[core_s, pos_in_stream] = dstl_s

    # one-hot tensor per core: [128, nchunk_core * DW]
    chunk_id = np.arange(n_stream) // P
    row_in_chunk = np.arange(n_stream) % P
    oh_mat = np.zeros((NCORES, P, nchunk_core * DW), dtype=np.float32)
    for c in range(NCORES):
        oh_mat[c, row_in_chunk, chunk_id * DW + dstl_all[c]] = oh_all[c]

    # idx tables wrapped for dma_gather: idx i -> partition i%16 (x8 groups),
    # column i//16
    idx_wrap = np.empty((NCORES, P, n_stream // 16), dtype=np.int16)
    for c in range(NCORES):
        w = idx_all[c].reshape(-1, 16).T  # [16, cols]
        idx_wrap[c] = np.tile(w, (8, 1))

    return {
        "slot": slot, "node_slot": node_slot, "CL": CL, "CH": CH,
        "idx_wrap": idx_wrap, "oh_mat": oh_mat, "nchunk_core": nchunk_core,
    }


def _build_bass(CL, CH, nchunk_core):
    import concourse.bacc as bacc
    import concourse.mybir as mybir
    import concourse.tile as tile

    CPG = CL + CH
    SLAB_CH = SLAB_G * CPG               # chunks per slab
    LOW_CH = SLAB_G * CL                 # low chunks per slab
    N_LOW_GATHER = (SLAB_G * CL * P) // GSZ
    N_HIGH_GATHER = (SLAB_G * CH * P) // GSZ
    CPGATHER = GSZ // P                  # chunks per gather (8)
    NT = NBLK // P                       # 50 node tiles per core slice

    nc = bacc.Bacc(None, num_devices=NCORES, num_swdge_queues=4)
    dt = mybir.dt

    xT_d = nc.dram_tensor("xT", [P, NBLK], dt.float32, kind="ExternalInput")
    idx_d = nc.dram_tensor("idx", [P, nchunk_core * 8], dt.int16,
                           kind="ExternalInput")
    oh_d = nc.dram_tensor("oh", [P, nchunk_core * DW], dt.float32,
                          kind="ExternalInput")
    w_d = [nc.dram_tensor(f"W{l}", [P, P], dt.float32, kind="ExternalInput")
           for l in (1, 2, 3)]
    b_d = [nc.dram_tensor(f"b{l}", [P, 1], dt.float32, kind="ExternalInput")
           for l in (1, 2, 3)]
    eye_d = nc.dram_tensor("eye", [P, P], dt.float32, kind="ExternalInput")
    out_d = nc.dram_tensor("out", [NBLK, P], dt.float32, kind="ExternalOutput")

    core_ids = list(range(NCORES))

    with tile.TileContext(nc) as tc:
        with (
            tc.tile_pool(name="cst", bufs=1) as cst,
            tc.tile_pool(name="msgs", bufs=2) as msgs,
            tc.tile_pool(name="ohp", bufs=2) as ohp,
            tc.tile_pool(name="htp", bufs=1) as htp,
            tc.tile_pool(name="gst", bufs=1) as gst,
            tc.tile_pool(name="accp", bufs=4, space="PSUM") as accp,
            tc.tile_pool(name="wps", bufs=3, space="PSUM") as wps,
            tc.tile_pool(name="dram", bufs=1, space="DRAM") as dram,
        ):
            idx_t = cst.tile([P, nchunk_core * 8], dt.int16)
            nc.sync.dma_start(idx_t[:], idx_d[:])
            w_t = [cst.tile([P, P], dt.float32, name=f"w{l}") for l in range(3)]
            b_t = [cst.tile([P, 1], dt.float32, name=f"bt{l}") for l in range(3)]
            for l in range(3):
                nc.sync.dma_start(w_t[l][:], w_d[l][:])
                nc.sync.dma_start(b_t[l][:], b_d[l][:])
            eye_t = cst.tile([P, P], dt.float32)
            nc.sync.dma_start(eye_t[:], eye_d[:])
            xT_t = cst.tile([P, NBLK], dt.float32)
            nc.sync.dma_start(xT_t[:], xT_d[:])

            # prime ACT engine clock on the bias loads (1-wait limit on HW)
            prime_t = cst.tile([P, 3], dt.float32)
            for l in range(3):
                nc.scalar.activation(prime_t[:, l : l + 1], b_t[l][:],
                                     mybir.ActivationFunctionType.Copy)

            g_full = [
                dram.tile([NTOT, P], dt.float32, name=f"gfull{l}",
                          addr_space="Shared")
                for l in range(3)
            ]
            g_slice = [
                dram.tile([NBLK, P], dt.float32, name=f"gslice{l}")
                for l in range(3)
            ]

            def produce_g(src_tiles, w_tile, layer):
                """node-major G_layer slice = (srcT tiles).T @ w, AG to full."""
                stage = gst.tile([P, NT, P], dt.float32, name="gstage")
                for j in range(NT):
                    ps = wps.tile([P, P], dt.float32, name="wps_t")
                    nc.tensor.matmul(ps[:], src_tiles[:, j * P : (j + 1) * P],
                                     w_tile[:], start=True, stop=True)
                    nc.scalar.activation(stage[:, j, :], ps[:],
                                         mybir.ActivationFunctionType.Copy)
                nc.sync.dma_start(
                    g_slice[layer][:].rearrange("(t p) f -> p t f", p=P),
                    stage[:],
                )
                nc.gpsimd.collective_compute(
                    "AllGather", mybir.AluOpType.bypass,
                    replica_groups=[core_ids],
                    ins=[g_slice[layer].opt()],
                    outs=[g_full[layer].opt()],
                )

            # pass 0: G_0 = x @ W1
            produce_g(xT_t[:], w_t[0][:], 0)

            for layer in range(NLAYER):
                h_t = htp.tile([P, NBLK], dt.float32, name="ht")
                g_lo = g_full[layer][0:SPLIT, :]
                g_hi = g_full[layer][SPLIT:NTOT, :]
                qi = 0
                for s in range(NSLAB):
                    msg_t = msgs.tile([P, SLAB_CH, P], dt.float32, name="msg")
                    oh_t = ohp.tile([P, SLAB_CH * DW], dt.float32, name="oht")
                    c0 = s * SLAB_CH * P // 16
                    for k in range(N_LOW_GATHER):
                        nc.gpsimd.dma_gather(
                            msg_t[:, k * CPGATHER : (k + 1) * CPGATHER, :],
                            g_lo,
                            idx_t[:, c0 + k * GSZ // 16 : c0 + (k + 1) * GSZ // 16],
                            GSZ, GSZ, P,
                            single_packet=True, queue_num=qi % 4,
                        )
                        qi += 1
                    c0h = c0 + LOW_CH * P // 16
                    for k in range(N_HIGH_GATHER):
                        nc.gpsimd.dma_gather(
                            msg_t[:, LOW_CH + k * CPGATHER : LOW_CH + (k + 1) * CPGATHER, :],
                            g_hi,
                            idx_t[:, c0h + k * GSZ // 16 : c0h + (k + 1) * GSZ // 16],
                            GSZ, GSZ, P,
                            single_packet=True, queue_num=qi % 4,
                        )
                        qi += 1
                    nc.sync.dma_start(
                        oh_t[:],
                        oh_d[:, s * SLAB_CH * DW : (s + 1) * SLAB_CH * DW],
                    )
                    for g in range(SLAB_G):
                        acc = accp.tile([P, DW], dt.float32, name="acc")
                        cis = [g * CL + k for k in range(CL)] + [
                            LOW_CH + g * CH + k for k in range(CH)
                        ]
                        for j, ci in enumerate(cis):
                            nc.tensor.matmul(
                                acc[:],
                                msg_t[:, ci, :],
                                oh_t[:, ci * DW : (ci + 1) * DW],
                                start=(j == 0),
                                stop=(j == CPG - 1),
                            )
                        gi = s * SLAB_G + g
                        nc.scalar.activation(
                            h_t[:, gi * DW : (gi + 1) * DW], acc[:],
                            mybir.ActivationFunctionType.Lrelu,
                            bias=b_t[layer][:], scale=1.0, alpha=ALPHA,
                        )
                if layer < NLAYER - 1:
                    produce_g(h_t[:], w_t[layer + 1][:], layer + 1)
                else:
                    stage = gst.tile([P, NT, P], dt.float32, name="gstage")
                    for j in range(NT):
                        ps = wps.tile([P, P], dt.float32, name="wps_t")
                        nc.tensor.matmul(ps[:], h_t[:, j * P : (j + 1) * P],
                                         eye_t[:], start=True, stop=True)
                        nc.scalar.activation(
                            stage[:, j, :], ps[:],
                            mybir.ActivationFunctionType.Copy)
                    nc.sync.dma_start(
                        out_d[:].rearrange("(t p) f -> p t f", p=P), stage[:]
                    )

    nc.finalize()
    return nc


_CACHE = {}


def build(x, edge_index):
    """Preprocess + build the bass program; cached on graph identity."""
    meta = _preprocess(np.asarray(x), np.asarray(edge_index))
    nc = _build_bass(meta["CL"], meta["CH"], meta["nchunk_core"])
    return nc, meta


def make_in_maps(x, W1, b1, W2, b2, W3, b3, meta):
    x = np.asarray(x, dtype=np.float32)
    slot = meta["slot"]
    x_slot = np.zeros((NTOT, P), dtype=np.float32)
    valid = slot >= 0
    x_slot[valid] = x[slot[valid]]
    eye = np.eye(P, dtype=np.float32)
    in_maps = []
    for c in range(NCORES):
        xT = np.ascontiguousarray(x_slot[c * NBLK : (c + 1) * NBLK].T)
        in_maps.append({
            "xT": xT,
            "idx": meta["idx_wrap"][c],
            "oh": np.ascontiguousarray(meta["oh_mat"][c]),
            "W1": np.asarray(W1, np.float32),
            "W2": np.asarray(W2, np.float32),
            "W3": np.asarray(W3, np.float32),
            "b1": np.asarray(b1, np.float32).reshape(P, 1),
            "b2": np.asarray(b2, np.float32).reshape(P, 1),
            "b3": np.asarray(b3, np.float32).reshape(P, 1),
            "eye": eye,
        })
    return in_maps


def assemble_output(results, meta, n_nodes):
    out_slots = np.concatenate([results[c]["out"] for c in range(NCORES)], axis=0)
    return np.ascontiguousarray(out_slots[meta["node_slot"][:n_nodes]])


def kernel(x, edge_index, W1, b1, W2, b2, W3, b3):
    from concourse.bass_utils import run_bass_kernel_spmd

    key = "k"
    if key not in _CACHE:
        nc, meta = build(x, edge_index)
        _CACHE[key] = (nc, meta)
    nc, meta = _CACHE[key]
    in_maps = make_in_maps(x, W1, b1, W2, b2, W3, b3, meta)
    res = run_bass_kernel_spmd(nc, in_maps, list(range(NCORES)))
    return assemble_output(res.results, meta, np.asarray(x).shape[0])


# revision 5
# speedup vs baseline: 1.0325x; 1.0325x over previous
"""GCN 3-layer message-passing kernel for Trainium2 (8 NeuronCores).

Strategy
--------
Nodes are sharded across the 8 cores by destination (graph parallel), with a
degree-balanced permutation so every 64-node "group" has a near-equal edge
count. Per layer, each core:
  1. gathers source-node feature rows (f32, 512B) from its local full copy of
     G = H @ W with `dma_gather` (single-packet, 1024 idx/call, 4 SWDGE
     queues — descriptor generation on the Q7 cores is the bottleneck),
  2. aggregates 128-edge chunks into PSUM via matmul against a host-built
     one-hot matrix carrying the symmetric norm (out = Msg.T @ OneHot,
     feature-major [128f x 64dst] accumulators),
  3. applies bias + LeakyReLU on the scalar engine (Lrelu activation),
  4. computes the next layer's G tiles node-major with a stationary trick
     (lhsT = H^T tile, rhs = W) and AllGathers the slice to every core.
The final layer multiplies by identity instead of W to emit node-major f32
output rows.
"""

import sys

for _p in ("/opt/trn_rl_repo", "/root/.axon_site"):
    if _p not in sys.path:
        sys.path.insert(0, _p)

import numpy as np

P = 128
DW = 64            # dst nodes per PSUM group
NCORES = 8
NBLK = 6400        # dst slots per core (100 groups of 64)
NG = NBLK // DW    # 100 groups per core
SLAB_G = 4         # groups per slab (gather/onehot batch)
NSLAB = NG // SLAB_G
NTOT = NBLK * NCORES  # 51200 slots
SPLIT = 32768      # int16 gather index split
GSZ = 1024         # idxs per dma_gather (single-packet limit)
ALPHA = 0.01
NLAYER = 3
AGG_BF16 = False


def _even_ceil(n, m):
    c = -(-n // m)
    return c + (c % 2)


def _preprocess(x, edge_index):
    """Host-side graph preprocessing -> per-core idx tables, onehot, layout."""
    n_nodes = x.shape[0]
    src = np.concatenate([edge_index[0], np.arange(n_nodes, dtype=np.int64)])
    dst = np.concatenate([edge_index[1], np.arange(n_nodes, dtype=np.int64)])
    deg = np.bincount(dst, minlength=n_nodes).astype(np.float64)
    dinv = np.where(deg > 0, deg**-0.5, 0.0)
    norm = (dinv[src] * dinv[dst]).astype(np.float32)

    # --- degree-balanced node -> slot assignment (snake deal into groups) ---
    n_groups_tot = NTOT // DW  # 800
    order = np.argsort(-deg, kind="stable")  # high degree first
    group_of = np.empty(NTOT, dtype=np.int64)  # by deal position
    pos = np.arange(NTOT)
    row = pos // n_groups_tot
    col = pos % n_groups_tot
    group_of = np.where(row % 2 == 0, col, n_groups_tot - 1 - col)
    # node order[i] -> group group_of[i]; remaining slots go to pad nodes
    slot = np.full(NTOT, -1, dtype=np.int64)  # slot -> node (-1 pad)
    node_slot = np.empty(n_nodes, dtype=np.int64)
    fill = np.zeros(n_groups_tot, dtype=np.int64)
    g_arr = group_of[:n_nodes]
    # sequential fill within groups (vector friendly: argsort by group, stable)
    order_by_group = np.argsort(g_arr, kind="stable")
    gs = g_arr[order_by_group]
    within = np.arange(n_nodes) - np.searchsorted(gs, gs)
    slots_for = gs * DW + within
    node_slot[order[order_by_group]] = slots_for
    slot[slots_for] = order[order_by_group]

    s_src = node_slot[src]
    s_dst = node_slot[dst]
    is_high = s_src >= SPLIT
    core = s_dst // NBLK
    group_g = s_dst // DW  # global group id
    dst_local = s_dst % DW

    # per (group, range) edge counts -> uniform CL/CH chunk counts
    key = group_g * 2 + is_high
    counts = np.bincount(key, minlength=n_groups_tot * 2)
    low_max = counts[0::2].max()
    high_max = counts[1::2].max()
    CL = _even_ceil(int(low_max), P)
    CH = _even_ceil(int(high_max), P)
    assert (SLAB_G * CL * P) % GSZ == 0 and (SLAB_G * CH * P) % GSZ == 0
    CPG = CL + CH
    nchunk_core = NG * CPG

    # order edges by (group, range, arbitrary) and compute padded positions
    sort_key = key
    eorder = np.argsort(sort_key, kind="stable")
    s_src_s = s_src[eorder]
    is_high_s = is_high[eorder]
    group_s = group_g[eorder]
    dstl_s = dst_local[eorder]
    norm_s = norm[eorder]
    # position within its (group, range) bucket
    k_s = sort_key[eorder]
    bucket_start = np.searchsorted(k_s, k_s)
    within_b = np.arange(len(k_s)) - bucket_start

    # padded slot id of each edge inside its core's layer-stream
    # stream layout per core: slab-major; per slab: 4 groups' low chunks
    # (each CL*128), then 4 groups' high chunks (each CH*128)
    g_in_core = group_s % (NG)
    slab = g_in_core // SLAB_G
    g_in_slab = g_in_core % SLAB_G
    slab_base = slab * SLAB_G * CPG * P
    off_range = np.where(is_high_s, SLAB_G * CL * P + g_in_slab * CH * P,
                         g_in_slab * CL * P)
    pos_in_stream = slab_base + off_range + within_b
    core_s = group_s // NG

    n_stream = nchunk_core * P
    idx_all = np.zeros((NCORES, n_stream), dtype=np.int16)
    oh_all = np.zeros((NCORES, n_stream), dtype=np.float32)  # norm per slot
    dstl_all = np.zeros((NCORES, n_stream), dtype=np.int64)
    gval = np.where(is_high_s, s_src_s - SPLIT, s_src_s).astype(np.int16)
    idx_all[core_s, pos_in_stream] = gval
    oh_all[core_s, pos_in_stream] = norm_s
    dstl_all[core_s, pos_in_stream] = dstl_s

    # one-hot tensor per core: [128, nchunk_core * DW]
    chunk_id = np.arange(n_stream) // P
    row_in_chunk = np.arange(n_stream) % P
    oh_mat = np.zeros((NCORES, P, nchunk_core * DW), dtype=np.float32)
    for c in range(NCORES):
        oh_mat[c, row_in_chunk, chunk_id * DW + dstl_all[c]] = oh_all[c]

    # idx tables wrapped for dma_gather: idx i -> partition i%16 (x8 groups),
    # column i//16
    idx_wrap = np.empty((NCORES, P, n_stream // 16), dtype=np.int16)
    for c in range(NCORES):
        w = idx_all[c].reshape(-1, 16).T  # [16, cols]
        idx_wrap[c] = np.tile(w, (8, 1))

    return {
        "slot": slot, "node_slot": node_slot, "CL": CL, "CH": CH,
        "idx_wrap": idx_wrap, "oh_mat": oh_mat, "nchunk_core": nchunk_core,
    }


def _build_bass(CL, CH, nchunk_core):
    import concourse.bacc as bacc
    import concourse.mybir as mybir
    import concourse.tile as tile

    CPG = CL + CH
    SLAB_CH = SLAB_G * CPG               # chunks per slab
    LOW_CH = SLAB_G * CL                 # low chunks per slab
    N_LOW_GATHER = (SLAB_G * CL * P) // GSZ
    N_HIGH_GATHER = (SLAB_G * CH * P) // GSZ
    CPGATHER = GSZ // P                  # chunks per gather (8)
    NT = NBLK // P                       # 50 node tiles per core slice

    nc = bacc.Bacc(None, num_devices=NCORES, num_swdge_queues=4)
    dt = mybir.dt

    xT_d = nc.dram_tensor("xT", [P, NBLK], dt.float32, kind="ExternalInput")
    idx_d = nc.dram_tensor("idx", [P, nchunk_core * 8], dt.int16,
                           kind="ExternalInput")
    dt_agg = dt.bfloat16 if AGG_BF16 else dt.float32
    oh_d = nc.dram_tensor("oh", [P, nchunk_core * DW], dt_agg,
                          kind="ExternalInput")
    w_d = [nc.dram_tensor(f"W{l}", [P, P], dt.float32, kind="ExternalInput")
           for l in (1, 2, 3)]
    b_d = [nc.dram_tensor(f"b{l}", [P, 1], dt.float32, kind="ExternalInput")
           for l in (1, 2, 3)]
    eye_d = nc.dram_tensor("eye", [P, P], dt.float32, kind="ExternalInput")
    out_d = nc.dram_tensor("out", [NBLK, P], dt.float32, kind="ExternalOutput")

    core_ids = list(range(NCORES))

    with tile.TileContext(nc) as tc:
        with (
            tc.tile_pool(name="cst", bufs=1) as cst,
            tc.tile_pool(name="msgs", bufs=3) as msgs,
            tc.tile_pool(name="ohp", bufs=3) as ohp,
            tc.tile_pool(name="htp", bufs=1) as htp,
            tc.tile_pool(name="gst", bufs=1) as gst,
            tc.tile_pool(name="accp", bufs=4, space="PSUM") as accp,
            tc.tile_pool(name="wps", bufs=3, space="PSUM") as wps,
            tc.tile_pool(name="dram", bufs=1, space="DRAM") as dram,
        ):
            idx_t = cst.tile([P, nchunk_core * 8], dt.int16)
            nc.sync.dma_start(idx_t[:], idx_d[:])
            w_t = [cst.tile([P, P], dt.float32, name=f"w{l}") for l in range(3)]
            b_t = [cst.tile([P, 1], dt.float32, name=f"bt{l}") for l in range(3)]
            for l in range(3):
                nc.sync.dma_start(w_t[l][:], w_d[l][:])
                nc.sync.dma_start(b_t[l][:], b_d[l][:])
            eye_t = cst.tile([P, P], dt.float32)
            nc.sync.dma_start(eye_t[:], eye_d[:])
            xT_t = cst.tile([P, NBLK], dt.float32)
            nc.sync.dma_start(xT_t[:], xT_d[:])

            # prime ACT engine clock on the bias loads (1-wait limit on HW)
            prime_t = cst.tile([P, 3], dt.float32)
            for l in range(3):
                nc.scalar.activation(prime_t[:, l : l + 1], b_t[l][:],
                                     mybir.ActivationFunctionType.Copy)

            g_full = [
                dram.tile([NTOT, P], dt_agg, name=f"gfull{l}",
                          addr_space="Shared")
                for l in range(3)
            ]
            g_slice = [
                dram.tile([NBLK, P], dt_agg, name=f"gslice{l}")
                for l in range(3)
            ]

            def produce_g(src_tiles, w_tile, layer):
                """node-major G_layer slice = (srcT tiles).T @ w, AG to full."""
                stage = gst.tile([P, NT, P], dt_agg, name="gstage")
                for j in range(NT):
                    ps = wps.tile([P, P], dt.float32, name="wps_t")
                    nc.tensor.matmul(ps[:], src_tiles[:, j * P : (j + 1) * P],
                                     w_tile[:], start=True, stop=True)
                    nc.scalar.activation(stage[:, j, :], ps[:],
                                         mybir.ActivationFunctionType.Copy)
                nc.sync.dma_start(
                    g_slice[layer][:].rearrange("(t p) f -> p t f", p=P),
                    stage[:],
                )
                nc.gpsimd.collective_compute(
                    "AllGather", mybir.AluOpType.bypass,
                    replica_groups=[core_ids],
                    ins=[g_slice[layer].opt()],
                    outs=[g_full[layer].opt()],
                )

            # pass 0: G_0 = x @ W1
            produce_g(xT_t[:], w_t[0][:], 0)

            for layer in range(NLAYER):
                h_t = htp.tile([P, NBLK], dt.float32, name="ht")
                g_lo = g_full[layer][0:SPLIT, :]
                g_hi = g_full[layer][SPLIT:NTOT, :]
                qi = 0
                for s in range(NSLAB):
                    msg_t = msgs.tile([P, SLAB_CH, P], dt_agg, name="msg")
                    oh_t = ohp.tile([P, SLAB_CH * DW], dt_agg, name="oht")
                    c0 = s * SLAB_CH * P // 16
                    for k in range(N_LOW_GATHER):
                        nc.gpsimd.dma_gather(
                            msg_t[:, k * CPGATHER : (k + 1) * CPGATHER, :],
                            g_lo,
                            idx_t[:, c0 + k * GSZ // 16 : c0 + (k + 1) * GSZ // 16],
                            GSZ, GSZ, P,
                            single_packet=True, queue_num=qi % 4,
                        )
                        qi += 1
                    c0h = c0 + LOW_CH * P // 16
                    for k in range(N_HIGH_GATHER):
                        nc.gpsimd.dma_gather(
                            msg_t[:, LOW_CH + k * CPGATHER : LOW_CH + (k + 1) * CPGATHER, :],
                            g_hi,
                            idx_t[:, c0h + k * GSZ // 16 : c0h + (k + 1) * GSZ // 16],
                            GSZ, GSZ, P,
                            single_packet=True, queue_num=qi % 4,
                        )
                        qi += 1
                    nc.sync.dma_start(
                        oh_t[:],
                        oh_d[:, s * SLAB_CH * DW : (s + 1) * SLAB_CH * DW],
                    )
                    for g in range(SLAB_G):
                        acc = accp.tile([P, DW], dt.float32, name="acc")
                        cis = [g * CL + k for k in range(CL)] + [
                            LOW_CH + g * CH + k for k in range(CH)
                        ]
                        for j, ci in enumerate(cis):
                            nc.tensor.matmul(
                                acc[:],
                                msg_t[:, ci, :],
                                oh_t[:, ci * DW : (ci + 1) * DW],
                                start=(j == 0),
                                stop=(j == CPG - 1),
                            )
                        gi = s * SLAB_G + g
                        nc.scalar.activation(
                            h_t[:, gi * DW : (gi + 1) * DW], acc[:],
                            mybir.ActivationFunctionType.Lrelu,
                            bias=b_t[layer][:], scale=1.0, alpha=ALPHA,
                        )
                if layer < NLAYER - 1:
                    produce_g(h_t[:], w_t[layer + 1][:], layer + 1)
                else:
                    stage = gst.tile([P, NT, P], dt.float32, name="gstage")
                    for j in range(NT):
                        ps = wps.tile([P, P], dt.float32, name="wps_t")
                        nc.tensor.matmul(ps[:], h_t[:, j * P : (j + 1) * P],
                                         eye_t[:], start=True, stop=True)
                        nc.scalar.activation(
                            stage[:, j, :], ps[:],
                            mybir.ActivationFunctionType.Copy)
                    nc.sync.dma_start(
                        out_d[:].rearrange("(t p) f -> p t f", p=P), stage[:]
                    )

    nc.finalize()
    return nc


_CACHE = {}


def build(x, edge_index):
    """Preprocess + build the bass program; cached on graph identity."""
    meta = _preprocess(np.asarray(x), np.asarray(edge_index))
    nc = _build_bass(meta["CL"], meta["CH"], meta["nchunk_core"])
    return nc, meta


def make_in_maps(x, W1, b1, W2, b2, W3, b3, meta):
    x = np.asarray(x, dtype=np.float32)
    slot = meta["slot"]
    x_slot = np.zeros((NTOT, P), dtype=np.float32)
    valid = slot >= 0
    x_slot[valid] = x[slot[valid]]
    eye = np.eye(P, dtype=np.float32)
    in_maps = []
    for c in range(NCORES):
        xT = np.ascontiguousarray(x_slot[c * NBLK : (c + 1) * NBLK].T)
        in_maps.append({
            "xT": xT,
            "idx": meta["idx_wrap"][c],
            "oh": (np.ascontiguousarray(meta["oh_mat"][c]).astype(__import__("ml_dtypes").bfloat16) if AGG_BF16 else np.ascontiguousarray(meta["oh_mat"][c])),
            "W1": np.asarray(W1, np.float32),
            "W2": np.asarray(W2, np.float32),
            "W3": np.asarray(W3, np.float32),
            "b1": np.asarray(b1, np.float32).reshape(P, 1),
            "b2": np.asarray(b2, np.float32).reshape(P, 1),
            "b3": np.asarray(b3, np.float32).reshape(P, 1),
            "eye": eye,
        })
    return in_maps


def assemble_output(results, meta, n_nodes):
    out_slots = np.concatenate([results[c]["out"] for c in range(NCORES)], axis=0)
    return np.ascontiguousarray(out_slots[meta["node_slot"][:n_nodes]])


def kernel(x, edge_index, W1, b1, W2, b2, W3, b3):
    from concourse.bass_utils import run_bass_kernel_spmd

    key = "k"
    if key not in _CACHE:
        nc, meta = build(x, edge_index)
        _CACHE[key] = (nc, meta)
    nc, meta = _CACHE[key]
    in_maps = make_in_maps(x, W1, b1, W2, b2, W3, b3, meta)
    res = run_bass_kernel_spmd(nc, in_maps, list(range(NCORES)))
    return assemble_output(res.results, meta, np.asarray(x).shape[0])
